# revision 1
# baseline (speedup 1.0000x reference)
"""MultiHeadEMA on 8 Trainium2 NeuronCores.

Strategy
--------
Channel-sharded: embed_dim=1024 -> 8 slices of 128 channels (= SBUF
partitions), one per core. The reference's FFT conv is exactly an order-2 IIR
    y_n[l] = q_n y_n[l-1] + x[l],   out = silu(c0 y0 + c1 y1 + omega x)
computed with `tensor_tensor_scan` on the vector engine.

The DVE scan runs at ~2.1 cyc/elem, so the recurrence is decimated by 4:
    Y_n[j] = y_n[4j] satisfies  Y_n[j] = q_n^4 Y_n[j-1] + u_n[j]
    u_n[j] = x[4j] + q_n x[4j-1] + q_n^2 x[4j-2] + q_n^3 x[4j-3]
u_n is built by accumulating diagonal matmuls (tensor engine, bf16) into
PSUM from contiguous phase blocks of x (deinterleaved and pre-shifted on the
host — a strided matmul rhs halves PE throughput). The scan reads u straight
from PSUM at 1/4 length. Phases y[4j+r] are never materialized: the outputs
    pre_r = c0 y0[4j+r] + c1 y1[4j+r] + w x[4j+r]
expand into diagonal matmuls over (Y0, Y1, phase blocks of x) with
per-channel coefficients (c_n q_n^r, sums), accumulated in PSUM, then one
Silu per 1024 columns evacuates PSUM -> SBUF (phase-major output, host
re-interleaves). Interior is bf16 (fp32 PSUM accumulation, fp32 scan state,
exact fp32 decay factors).

Engine balance at 8 concurrent cores: the chip power governor caps matmuls
at ~379ns (vs 216ns single-core) and punishes load added to the vector
engine (scans degrade), so the design keeps DVE scan-only and feeds the
tensor engine one dense contiguous stream.
"""

import numpy as np
import ml_dtypes

import concourse.bass as bass
import concourse.bacc as bacc
import concourse.tile as tile
from concourse import mybir
from concourse.bass_utils import run_bass_kernel_spmd

SEQ_LEN, BSZ, EMBED_DIM, NDIM = 4096, 4, 1024, 2
N_CORES = 8
D_PER = EMBED_DIM // N_CORES  # 128 channels/core = full SBUF partitions
SCALE = (1.0 / NDIM) ** 0.5
DEC = 4                   # decimation factor
J = SEQ_LEN // DEC        # decimated length 1024
CH = 512                  # matmul chunk (one fp32 PSUM bank)
NG = J // CH              # j-groups per slab (2)
F32 = mybir.dt.float32
BF16 = mybir.dt.bfloat16
AF = mybir.ActivationFunctionType
ALU = mybir.AluOpType

# x phase blocks: r = 0..3 -> x[4j+r]. Shifted u-operands x[4j-k] are read
# as contiguous offset-(-1) views of block (4-k); only STRIDED rhs is slow.
NBLK = 4


def build_bass():
    nc = bacc.Bacc(name="multihead_ema")
    x = nc.dram_tensor("x", [D_PER, BSZ, NBLK, J], BF16, kind="ExternalInput")
    # coef columns: [delta0, delta1, alpha0, alpha1, beta0, beta1, gamma0, gamma1, omega]
    coef = nc.dram_tensor("coef", [D_PER, 9], F32, kind="ExternalInput")
    eye = nc.dram_tensor("eye", [D_PER, D_PER], BF16, kind="ExternalInput")
    out = nc.dram_tensor("out", [D_PER, BSZ, DEC, J], BF16, kind="ExternalOutput")

    with tile.TileContext(nc) as tc:
        with (
            tc.tile_pool(name="const", bufs=1) as const,
            tc.tile_pool(name="xup", bufs=4) as xup,
            tc.tile_pool(name="xcp", bufs=4) as xcp,
            tc.tile_pool(name="yp", bufs=3) as yp,
            tc.tile_pool(name="op", bufs=4) as op,
            tc.tile_pool(name="ysp", bufs=3) as ysp,
            tc.tile_pool(name="psu", bufs=2, space="PSUM") as psu,
            tc.tile_pool(name="psc", bufs=2, space="PSUM") as psc,
        ):
            csb = const.tile([D_PER, 9], F32)
            nc.sync.dma_start(out=csb[:, :], in_=coef[:, :])
            eyesb = const.tile([D_PER, D_PER], BF16)
            nc.sync.dma_start(out=eyesb[:, :], in_=eye[:, :])

            # --- per-channel coefficients ([128, 1/2] fp32, trivial)
            sig = const.tile([D_PER, 4], F32)  # [p0, p1, sa0, sa1]
            nc.scalar.activation(out=sig[:, :], in_=csb[:, 0:4], func=AF.Sigmoid)
            pq = const.tile([D_PER, NDIM], F32)
            nc.vector.tensor_mul(out=pq[:, :], in0=sig[:, 0:2], in1=sig[:, 2:4])
            q = const.tile([D_PER, NDIM], F32)  # q = 1 - p*sigmoid(alpha)
            nc.scalar.activation(out=q[:, :], in_=pq[:, :], func=AF.Copy,
                                 scale=-1.0, bias=1.0)
            q2 = const.tile([D_PER, NDIM], F32)
            nc.vector.tensor_mul(out=q2[:, :], in0=q[:, :], in1=q[:, :])
            q3 = const.tile([D_PER, NDIM], F32)
            nc.vector.tensor_mul(out=q3[:, :], in0=q2[:, :], in1=q[:, :])
            q4 = const.tile([D_PER, NDIM], F32)
            nc.vector.tensor_mul(out=q4[:, :], in0=q2[:, :], in1=q2[:, :])
            c1t = const.tile([D_PER, NDIM], F32)
            nc.vector.tensor_mul(out=c1t[:, :], in0=sig[:, 0:2], in1=csb[:, 4:6])
            c2t = const.tile([D_PER, NDIM], F32)
            nc.vector.tensor_mul(out=c2t[:, :], in0=c1t[:, :], in1=csb[:, 6:8])
            cc = const.tile([D_PER, NDIM], F32)  # c_n = p beta gamma scale
            nc.scalar.mul(out=cc[:, :], in_=c2t[:, :], mul=SCALE)
            cq = const.tile([D_PER, NDIM], F32)   # c_n q_n
            nc.vector.tensor_mul(out=cq[:, :], in0=cc[:, :], in1=q[:, :])
            cq2 = const.tile([D_PER, NDIM], F32)  # c_n q_n^2
            nc.vector.tensor_mul(out=cq2[:, :], in0=cc[:, :], in1=q2[:, :])
            cq3 = const.tile([D_PER, NDIM], F32)  # c_n q_n^3
            nc.vector.tensor_mul(out=cq3[:, :], in0=cc[:, :], in1=q3[:, :])
            csum = const.tile([D_PER, 1], F32)    # c0 + c1 + w
            nc.vector.tensor_add(out=csum[:, :], in0=cc[:, 0:1], in1=cc[:, 1:2])
            nc.vector.tensor_add(out=csum[:, :], in0=csum[:, :], in1=csb[:, 8:9])
            cqs = const.tile([D_PER, 1], F32)     # c0 q0 + c1 q1
            nc.vector.tensor_add(out=cqs[:, :], in0=cq[:, 0:1], in1=cq[:, 1:2])
            cq2s = const.tile([D_PER, 1], F32)    # c0 q0^2 + c1 q1^2
            nc.vector.tensor_add(out=cq2s[:, :], in0=cq2[:, 0:1], in1=cq2[:, 1:2])

            # --- bf16 diagonal weight matrices
            _dn = [0]

            def diag(scalar_ap):
                _dn[0] += 1
                t = const.tile([D_PER, D_PER], BF16, tag=f"diag{_dn[0]}")
                nc.vector.tensor_scalar_mul(out=t[:, :], in0=eyesb[:, :],
                                            scalar1=scalar_ap)
                return t

            w_q = [[diag(t[:, n : n + 1]) for n in range(NDIM)] for t in (q, q2, q3)]
            w_cy = [[diag(t[:, n : n + 1]) for n in range(NDIM)]
                    for t in (cc, cq, cq2, cq3)]  # Y-term weights for r=0..3
            w_w = diag(csb[:, 8:9])    # x term of pre_0
            w_cw = diag(csum[:, 0:1])  # x_pr self term, r>=1
            w_cqs = diag(cqs[:, 0:1])
            w_cq2s = diag(cq2s[:, 0:1])

            q4b = [q4[:, n : n + 1].to_broadcast([D_PER, J]) for n in range(NDIM)]

            # prefetch all slabs; u-blocks in their own (earlier) transfers so
            # the first matmuls are gated by a 1MB DMA instead of 1.75MB
            xus = []
            for b in range(BSZ):
                xu = xup.tile([D_PER, 4, J], BF16, tag="xu")
                nc.sync.dma_start(out=xu[:, :, :], in_=x[:, b, :, :])
                xus.append(xu)

            for b in range(BSZ):
                xu = xus[b]

                # --- u_n in PSUM, Y_n = scan(q_n^4, u_n)
                Y = []
                for n in range(NDIM):
                    pu = psu.tile([D_PER, J], F32, tag="u")
                    for g in range(NG):
                        s = bass.ts(g, CH)
                        # c_n is folded into u: scan output is Y'_n = c_n Y_n
                        nc.tensor.matmul(pu[:, s], w_cy[0][n][:, :], xu[:, 0, s],
                                         start=True, stop=False)
                        for k in range(1, 4):  # + c_n q^k * x[4j-k]
                            if g == 0:
                                nc.tensor.matmul(
                                    pu[:, 1:CH], w_cy[k][n][:, :],
                                    xu[:, 4 - k, 0 : CH - 1],
                                    start=False, stop=(k == 3))
                            else:
                                nc.tensor.matmul(
                                    pu[:, s], w_cy[k][n][:, :],
                                    xu[:, 4 - k, g * CH - 1 : (g + 1) * CH - 1],
                                    start=False, stop=(k == 3))
                    yn = yp.tile([D_PER, J], BF16, tag=f"y{n}")
                    nc.vector.tensor_tensor_scan(
                        out=yn[:, :], data0=q4b[n], data1=pu[:, :],
                        initial=0.0, op0=ALU.mult, op1=ALU.add,
                    )
                    Y.append(yn)

                # --- outputs: pre_r accumulated in PSUM, silu evacuates
                ob = op.tile([D_PER, DEC, J], BF16)
                for pair in (1, 0):  # heavy phase-pair (2,3) first: lighter tail
                    for g in range(NG):
                        s = bass.ts(g, CH)
                        pt = psc.tile([D_PER, 2 * CH], F32, tag="cmb")
                        for h in range(2):
                            r = 2 * pair + h
                            tgt = pt[:, bass.ts(h, CH)]
                            if r == 0:
                                ysum = ysp.tile([D_PER, CH], BF16, tag="ys")
                                nc.vector.tensor_add(out=ysum[:, :],
                                                     in0=Y[0][:, s], in1=Y[1][:, s])
                                nc.tensor.matmul(tgt, eyesb[:, :], ysum[:, :],
                                                 start=True, stop=False)
                            else:
                                nc.tensor.matmul(tgt, w_q[r - 1][0][:, :], Y[0][:, s],
                                                 start=True, stop=False)
                                nc.tensor.matmul(tgt, w_q[r - 1][1][:, :], Y[1][:, s],
                                                 start=False, stop=False)
                            # x terms: phase r block is xc[r-1] (r>=1), xu[0] for r=0
                            xw = [(w_w, None) if r == 0 else (w_cw, r)]
                            if r == 2:
                                xw.append((w_cqs, 1))
                            elif r == 3:
                                xw.append((w_cqs, 2))
                                xw.append((w_cq2s, 1))
                            for i, (wt, rr) in enumerate(xw):
                                rhs = xu[:, 0, s] if rr is None else xu[:, rr, s]
                                nc.tensor.matmul(tgt, wt[:, :], rhs,
                                                 start=False, stop=(i == len(xw) - 1))
                        # silu: pt[:, h*512 + k] -> ob[:, 2*pair + h, 512g + k]
                        in_ap = pt[:, :].rearrange("p (h k) -> p h k", h=2)
                        nc.scalar.activation(
                            out=ob[:, 2 * pair : 2 * pair + 2, s],
                            in_=in_ap, func=AF.Silu)
                    # stream this phase-pair out while the next pair computes
                    nc.sync.dma_start(
                        out=out[:, b, 2 * pair : 2 * pair + 2, :],
                        in_=ob[:, 2 * pair : 2 * pair + 2, :])

    nc.compile()
    return nc


_CACHE: dict = {}


def _get_nc():
    if "nc" not in _CACHE:
        _CACHE["nc"] = build_bass()
    return _CACHE["nc"]


def make_in_maps(inputs):
    x = np.asarray(inputs["x"], np.float32)
    delta = np.asarray(inputs["delta"], np.float32).reshape(EMBED_DIM, NDIM)
    alpha = np.asarray(inputs["alpha"], np.float32).reshape(EMBED_DIM, NDIM)
    beta = np.asarray(inputs["beta"], np.float32).reshape(EMBED_DIM, NDIM)
    gamma = np.asarray(inputs["gamma"], np.float32).reshape(EMBED_DIM, NDIM)
    omega = np.asarray(inputs["omega"], np.float32).reshape(EMBED_DIM, 1)
    coef_full = np.concatenate([delta, alpha, beta, gamma, omega], axis=1)
    eye = np.eye(D_PER, dtype=ml_dtypes.bfloat16)
    in_maps = []
    for c in range(N_CORES):
        sl = slice(c * D_PER, (c + 1) * D_PER)
        xc = x[:, :, sl].transpose(2, 1, 0).astype(ml_dtypes.bfloat16)  # [128,B,L]
        ph = xc.reshape(D_PER, BSZ, J, DEC).transpose(0, 1, 3, 2)  # [128,B,4,J]
        in_maps.append(
            {"x": np.ascontiguousarray(ph),
             "coef": np.ascontiguousarray(coef_full[sl]), "eye": eye}
        )
    return in_maps


def gather_out(results):
    out = np.empty((SEQ_LEN, BSZ, EMBED_DIM), np.float32)
    for c in range(N_CORES):
        # [128, B, 4, J] phase-major -> [l = 4j+r, b, d]
        arr = results[c]["out"].astype(np.float32)
        out[:, :, c * D_PER : (c + 1) * D_PER] = arr.transpose(3, 2, 1, 0).reshape(
            SEQ_LEN, BSZ, D_PER
        )
    return out


def _run(inputs, **kwargs):
    nc = _get_nc()
    in_maps = make_in_maps(inputs)
    res = run_bass_kernel_spmd(nc, in_maps, core_ids=list(range(N_CORES)), **kwargs)
    return gather_out(res.results), res


def kernel(**inputs) -> np.ndarray:
    out, _ = _run(inputs)
    return out



# revision 2
# speedup vs baseline: 1.0870x; 1.0870x over previous
"""MultiHeadEMA on 8 Trainium2 NeuronCores.

Strategy
--------
Channel-sharded: embed_dim=1024 -> 8 slices of 128 channels (= SBUF
partitions), one per core. The reference's FFT conv is exactly an order-2 IIR
    y_n[l] = q_n y_n[l-1] + x[l],   out = silu(c0 y0 + c1 y1 + omega x)
computed with `tensor_tensor_scan` on the vector engine, decimated by 4:
    Y_n[j] = y_n[4j],  Y_n[j] = q_n^4 Y_n[j-1] + u_n[j]
    u_n[j] = x[4j] + q_n x[4j-1] + q_n^2 x[4j-2] + q_n^3 x[4j-3]
u_n is built by accumulating diagonal matmuls (tensor engine, bf16) into PSUM
from contiguous phase blocks of x (deinterleaved / pre-shifted on the host).
Phase outputs r>=1 expand into diagonal matmuls over (Y0, Y1, phase blocks)
accumulated in PSUM, evacuated by Silu on the scalar engine. Phase 0
(out_0 = Y'0 + Y'1 + w x0) is assembled on the vector engine in bf16 and
silu'd straight from SBUF, keeping it off the tensor engine.

v2 scheduling (the v1 kernel measured 67 us; PE streams 512-col bf16 MMs at
216 ns warm):
  * host precomputes the 20 per-channel coefficient columns -> no on-device
    sigmoid chain on the critical path; diag weights built on DVE (u-diags)
    and ACT (out-diags) in parallel right after a tiny DMA.
  * dummy warm-up matmuls on zeroed tiles run during the input-DMA head so
    the PE HAM clock-gate is already at 8/8 when real MMs start.
  * batch-0 input DMA split in half so u-matmuls start ~2 us earlier.
  * software pipelining: u(b+1) matmuls are interleaved between out(b)
    phase-group matmuls, hiding the DVE scan latency; scans are chained in
    512-col halves so Y becomes available earlier.
  * last batch evacuates PSUM with per-bank silus + small DMAs for a short
    tail.
"""

import numpy as np
import ml_dtypes

import concourse.bass as bass
import concourse.bacc as bacc
import concourse.tile as tile
from concourse import mybir
from concourse.bass_utils import run_bass_kernel_spmd

SEQ_LEN, BSZ, EMBED_DIM, NDIM = 4096, 4, 1024, 2
N_CORES = 8
D_PER = EMBED_DIM // N_CORES  # 128 channels/core = full SBUF partitions
SCALE = (1.0 / NDIM) ** 0.5
DEC = 4                   # decimation factor
J = SEQ_LEN // DEC        # decimated length 1024
CH = 512                  # matmul chunk (one fp32 PSUM bank)
NG = J // CH              # j-groups per slab (2)
F32 = mybir.dt.float32
BF16 = mybir.dt.bfloat16
AF = mybir.ActivationFunctionType
ALU = mybir.AluOpType
NBLK = 4                  # x phase blocks r=0..3 -> x[4j+r]
NWARM = 9                 # HAM warm-up matmuls

# coef columns (host precomputed, fp32):
#  0: q4_0   1: q4_1    scan decay q_n^4
#  2: cc_0   3: cc_1    c_n = p beta gamma scale   (u tap 0; c folded into u)
#  4: cq_0   5: cq_1    c_n q_n                    (u tap 1)
#  6: cq2_0  7: cq2_1   c_n q_n^2                  (u tap 2)
#  8: cq3_0  9: cq3_1   c_n q_n^3                  (u tap 3)
# 10: q_0   11: q_1     out r=1 Y weights
# 12: q2_0  13: q2_1    out r=2 Y weights
# 14: q3_0  15: q3_1    out r=3 Y weights
# 16: w                 omega (r=0 x term, DVE tensor_scalar)
# 17: csum = c0+c1+w    out r>=1 x self term
# 18: cqs  = c0q0+c1q1
# 19: cq2s = c0q0^2+c1q1^2
NCOEF = 20


def build_bass():
    nc = bacc.Bacc(name="multihead_ema_v2")
    x = nc.dram_tensor("x", [D_PER, BSZ, NBLK, J], BF16, kind="ExternalInput")
    coef = nc.dram_tensor("coef", [D_PER, NCOEF], F32, kind="ExternalInput")
    eye = nc.dram_tensor("eye", [D_PER, D_PER], BF16, kind="ExternalInput")
    out = nc.dram_tensor("out", [D_PER, BSZ, DEC, J], BF16, kind="ExternalOutput")

    with tile.TileContext(nc) as tc:
        with (
            tc.tile_pool(name="const", bufs=1) as const,
            tc.tile_pool(name="xup", bufs=4) as xup,
            tc.tile_pool(name="yp", bufs=2) as yp,
            tc.tile_pool(name="r0p", bufs=2) as r0p,
            tc.tile_pool(name="op", bufs=2) as op,
            tc.tile_pool(name="psu", bufs=2, space="PSUM") as psu,
            tc.tile_pool(name="psc", bufs=1, space="PSUM") as psc,
            tc.tile_pool(name="psw", bufs=1, space="PSUM") as psw,
        ):
            # --- warm-up operands (memset on gpsimd; no DMA dependency)
            wz = const.tile([D_PER, D_PER], BF16, tag="wz")
            nc.gpsimd.memset(wz[:, :], 0)
            mz = const.tile([D_PER, CH], BF16, tag="mz")
            nc.gpsimd.memset(mz[:, :], 0)

            # --- input DMAs (coef/eye tiny; xu b=0 split in halves)
            csb = const.tile([D_PER, NCOEF], F32)
            nc.sync.dma_start(out=csb[:, :], in_=coef[:, :])
            eyesb = const.tile([D_PER, D_PER], BF16)
            nc.sync.dma_start(out=eyesb[:, :], in_=eye[:, :])
            xus = []
            for b in range(BSZ):
                xu = xup.tile([D_PER, NBLK, J], BF16, tag="xu")
                if b == 0:
                    nc.sync.dma_start(out=xu[:, :, 0:CH], in_=x[:, b, :, 0:CH])
                    nc.sync.dma_start(out=xu[:, :, CH:J], in_=x[:, b, :, CH:J])
                else:
                    nc.sync.dma_start(out=xu[:, :, :], in_=x[:, b, :, :])
                xus.append(xu)

            # --- HAM warm-up: one accumulation group of dummy matmuls
            wps = psw.tile([D_PER, CH], F32)
            for i in range(NWARM):
                nc.tensor.matmul(wps[:, :], wz[:, :], mz[:, :],
                                 start=(i == 0), stop=(i == NWARM - 1))

            # --- diag weight matrices: u-diags on DVE, out-diags on ACT
            def diag_dve(col, tg):
                t = const.tile([D_PER, D_PER], BF16, tag=tg)
                nc.vector.tensor_scalar_mul(out=t[:, :], in0=eyesb[:, :],
                                            scalar1=csb[:, col : col + 1])
                return t

            def diag_act(col, tg):
                t = const.tile([D_PER, D_PER], BF16, tag=tg)
                nc.scalar.activation(out=t[:, :], in_=eyesb[:, :], func=AF.Copy,
                                     scale=csb[:, col : col + 1])
                return t

            # u-synthesis weights, ordered by first use (n=0 taps, n=1 taps)
            w_u = [[None] * 4, [None] * 4]
            for n in range(NDIM):
                for k in range(4):
                    w_u[n][k] = diag_dve(2 + 2 * k + n, f"wu{n}{k}")
            # out-stage weights, ordered by first use in outg
            w_q = [[None] * NDIM for _ in range(3)]  # [r-1][n] : diag(q_n^r)
            w_q[0][0] = diag_act(10, "wq10")
            w_q[0][1] = diag_act(11, "wq11")
            w_cw = diag_act(17, "wcw")    # csum
            w_q[1][0] = diag_act(12, "wq20")
            w_q[1][1] = diag_act(13, "wq21")
            w_cqs = diag_act(18, "wcqs")
            w_q[2][0] = diag_act(14, "wq30")
            w_q[2][1] = diag_act(15, "wq31")
            w_cq2s = diag_act(19, "wcq2s")

            q4b = [csb[:, n : n + 1].to_broadcast([D_PER, CH]) for n in range(NDIM)]

            pus = {}   # (b, n) -> psum tile
            Ys = {}    # (b, n) -> sbuf bf16 tile
            obs = {}   # b -> output tile

            def issue_u(b, n):
                """8 diagonal matmuls accumulating u_n for batch b into PSUM."""
                xu = xus[b]
                pu = psu.tile([D_PER, J], F32, tag="u")
                pus[(b, n)] = pu
                for g in range(NG):
                    s = bass.ts(g, CH)
                    nc.tensor.matmul(pu[:, s], w_u[n][0][:, :], xu[:, 0, s],
                                     start=True, stop=False)
                    for k in range(1, 4):  # + c_n q^k * x[4j-k]
                        if g == 0:
                            nc.tensor.matmul(
                                pu[:, 1:CH], w_u[n][k][:, :],
                                xu[:, 4 - k, 0 : CH - 1],
                                start=False, stop=(k == 3))
                        else:
                            nc.tensor.matmul(
                                pu[:, s], w_u[n][k][:, :],
                                xu[:, 4 - k, g * CH - 1 : (g + 1) * CH - 1],
                                start=False, stop=(k == 3))

            def issue_scan(b, n):
                """Chained half scans: Y'_n available per 512-col half."""
                pu = pus.pop((b, n))
                yn = yp.tile([D_PER, J], BF16, tag=f"y{n}")
                Ys[(b, n)] = yn
                nc.vector.tensor_tensor_scan(
                    out=yn[:, 0:CH], data0=q4b[n], data1=pu[:, 0:CH],
                    initial=0.0, op0=ALU.mult, op1=ALU.add)
                nc.vector.tensor_tensor_scan(
                    out=yn[:, CH:J], data0=q4b[n], data1=pu[:, CH:J],
                    initial=yn[:, CH - 1 : CH], op0=ALU.mult, op1=ALU.add)

            def issue_r0(b):
                """Phase 0 fully off the tensor engine:
                out_0 = silu(Y'0 + Y'1 + w*x0), assembled in bf16 on DVE."""
                ob = op.tile([D_PER, DEC, J], BF16)
                obs[b] = ob
                ysum = r0p.tile([D_PER, J], BF16, tag="ys")
                nc.vector.tensor_add(out=ysum[:, :], in0=Ys[(b, 0)][:, :],
                                     in1=Ys[(b, 1)][:, :])
                xw = r0p.tile([D_PER, J], BF16, tag="xw")
                nc.vector.tensor_scalar_mul(out=xw[:, :], in0=xus[b][:, 0, :],
                                            scalar1=csb[:, 16:17])
                pre0 = r0p.tile([D_PER, J], BF16, tag="p0")
                nc.vector.tensor_add(out=pre0[:, :], in0=ysum[:, :], in1=xw[:, :])
                nc.scalar.activation(out=ob[:, 0, :], in_=pre0[:, :], func=AF.Silu)
                nc.sync.dma_start(out=out[:, b, 0, :], in_=ob[:, 0, :])

            def issue_outg(b, g, split_silu):
                """Phases 1..3 for j-group g: 12 diag matmuls into 3 PSUM
                banks, silu evacuation, output DMA."""
                xu = xus[b]
                s = bass.ts(g, CH)
                pt = psc.tile([D_PER, 3, CH], F32, tag="cmb")
                for r in (1, 2, 3):
                    tgt = pt[:, r - 1, :]
                    nc.tensor.matmul(tgt, w_q[r - 1][0][:, :], Ys[(b, 0)][:, s],
                                     start=True, stop=False)
                    nc.tensor.matmul(tgt, w_q[r - 1][1][:, :], Ys[(b, 1)][:, s],
                                     start=False, stop=False)
                    xw = [(w_cw, r)]
                    if r == 2:
                        xw.append((w_cqs, 1))
                    elif r == 3:
                        xw.append((w_cqs, 2))
                        xw.append((w_cq2s, 1))
                    for i, (wt, rr) in enumerate(xw):
                        nc.tensor.matmul(tgt, wt[:, :], xu[:, rr, s],
                                         start=False, stop=(i == len(xw) - 1))
                    if split_silu:
                        nc.scalar.activation(out=obs[b][:, r, s],
                                             in_=pt[:, r - 1, :], func=AF.Silu)
                        nc.sync.dma_start(out=out[:, b, r, s],
                                          in_=obs[b][:, r, s])
                if not split_silu:
                    nc.scalar.activation(out=obs[b][:, 1:4, s],
                                         in_=pt[:, :, :], func=AF.Silu)
                    nc.sync.dma_start(out=out[:, b, 1:4, s],
                                      in_=obs[b][:, 1:4, s])

            # --- software-pipelined main loop
            issue_u(0, 0)
            issue_scan(0, 0)
            issue_u(0, 1)
            issue_scan(0, 1)
            issue_r0(0)
            for b in range(BSZ):
                last = b == BSZ - 1
                if not last:
                    issue_u(b + 1, 0)
                    issue_scan(b + 1, 0)
                issue_outg(b, 0, split_silu=last)
                if not last:
                    issue_u(b + 1, 1)
                    issue_scan(b + 1, 1)
                issue_outg(b, 1, split_silu=last)
                if not last:
                    issue_r0(b + 1)

    nc.compile()
    return nc


_CACHE: dict = {}


def _get_nc():
    if "nc" not in _CACHE:
        _CACHE["nc"] = build_bass()
    return _CACHE["nc"]


def _sigmoid64(a):
    return 1.0 / (1.0 + np.exp(-a.astype(np.float64)))


def make_in_maps(inputs):
    x = np.asarray(inputs["x"], np.float32)
    delta = np.asarray(inputs["delta"], np.float32).reshape(EMBED_DIM, NDIM)
    alpha = np.asarray(inputs["alpha"], np.float32).reshape(EMBED_DIM, NDIM)
    beta = np.asarray(inputs["beta"], np.float32).reshape(EMBED_DIM, NDIM)
    gamma = np.asarray(inputs["gamma"], np.float32).reshape(EMBED_DIM, NDIM)
    omega = np.asarray(inputs["omega"], np.float32).reshape(EMBED_DIM, 1)

    # per-channel coefficients (host precompute = weight repacking)
    p = _sigmoid64(delta)
    q = 1.0 - p * _sigmoid64(alpha)                      # [D, N] f64
    cc = p * beta.astype(np.float64) * gamma.astype(np.float64) * SCALE
    cols = np.empty((EMBED_DIM, NCOEF), np.float64)
    cols[:, 0:2] = q ** 4
    cols[:, 2:4] = cc
    cols[:, 4:6] = cc * q
    cols[:, 6:8] = cc * q ** 2
    cols[:, 8:10] = cc * q ** 3
    cols[:, 10:12] = q
    cols[:, 12:14] = q ** 2
    cols[:, 14:16] = q ** 3
    cols[:, 16:17] = omega
    cols[:, 17:18] = cc.sum(1, keepdims=True) + omega
    cols[:, 18:19] = (cc * q).sum(1, keepdims=True)
    cols[:, 19:20] = (cc * q ** 2).sum(1, keepdims=True)
    coef_full = cols.astype(np.float32)

    eye = np.eye(D_PER, dtype=ml_dtypes.bfloat16)
    in_maps = []
    for c in range(N_CORES):
        sl = slice(c * D_PER, (c + 1) * D_PER)
        xc = x[:, :, sl].transpose(2, 1, 0).astype(ml_dtypes.bfloat16)  # [128,B,L]
        ph = xc.reshape(D_PER, BSZ, J, DEC).transpose(0, 1, 3, 2)  # [128,B,4,J]
        in_maps.append(
            {"x": np.ascontiguousarray(ph),
             "coef": np.ascontiguousarray(coef_full[sl]), "eye": eye}
        )
    return in_maps


def gather_out(results):
    out = np.empty((SEQ_LEN, BSZ, EMBED_DIM), np.float32)
    for c in range(N_CORES):
        # [128, B, 4, J] phase-major -> [l = 4j+r, b, d]
        arr = results[c]["out"].astype(np.float32)
        out[:, :, c * D_PER : (c + 1) * D_PER] = arr.transpose(3, 2, 1, 0).reshape(
            SEQ_LEN, BSZ, D_PER
        )
    return out


def _run(inputs, **kwargs):
    nc = _get_nc()
    in_maps = make_in_maps(inputs)
    res = run_bass_kernel_spmd(nc, in_maps, core_ids=list(range(N_CORES)), **kwargs)
    return gather_out(res.results), res


def kernel(**inputs) -> np.ndarray:
    out, _ = _run(inputs)
    return out


# revision 3
# speedup vs baseline: 1.1972x; 1.1014x over previous
"""MultiHeadEMA on 8 Trainium2 NeuronCores.

Strategy
--------
Channel-sharded: embed_dim=1024 -> 8 slices of 128 channels (= SBUF
partitions), one per core. The reference's FFT conv is exactly an order-2 IIR
    y_n[l] = q_n y_n[l-1] + x[l],   out = silu(c0 y0 + c1 y1 + omega x)
computed with `tensor_tensor_scan` on the vector engine, decimated by 4:
    Y_n[j] = y_n[4j],  Y_n[j] = q_n^4 Y_n[j-1] + u_n[j]
    u_n[j] = x[4j] + q_n x[4j-1] + q_n^2 x[4j-2] + q_n^3 x[4j-3]
u_n is built by accumulating diagonal matmuls (tensor engine, bf16) into PSUM
from contiguous phase blocks of x (deinterleaved / pre-shifted on the host).
Phase outputs r>=1 expand into diagonal matmuls over (Y0, Y1, phase blocks)
accumulated in PSUM, evacuated by Silu on the scalar engine. Phase 0
(out_0 = Y'0 + Y'1 + w x0) is assembled on the vector engine in bf16 and
silu'd straight from SBUF, keeping it off the tensor engine.

v3 scheduling (v1 67.2us, v2 61.9us; PE streams 512-col bf16 MMs at 216 ns
warm so the whole game is keeping the MM stream dense and starting it early):
  * host precomputes the 20 per-channel coefficient columns; coef+eye ship
    as ONE small DMA (bitcast view) so diag-weight building starts ~+9us.
  * u-diags built on DVE, out-diags on ACT, in parallel.
  * dummy warm-up matmuls on zeroed tiles run during the DMA head so the PE
    HAM clock-gate is at 8/8 when real MMs start.
  * batch-0 x DMA split in halves, first half issued on the Scalar HWDGE
    queue in parallel with the const DMA on the Sync queue.
  * software pipelining: u(b+1) matmuls interleave between out(b) phase
    groups; scans are chained 512-col halves so Y is available early.
  * PSUM is split into per-bank tiles (Tile's dependency tracker is
    tile-granular): 4x u half-banks, a 2-bank (r1,r2) tile and a 1-bank r3
    tile, so silu evacuation never stalls the next matmul group.
"""

import numpy as np
import ml_dtypes

import concourse.bass as bass
import concourse.bacc as bacc
import concourse.tile as tile
from concourse import mybir
from concourse.bass_utils import run_bass_kernel_spmd

SEQ_LEN, BSZ, EMBED_DIM, NDIM = 4096, 4, 1024, 2
N_CORES = 8
D_PER = EMBED_DIM // N_CORES  # 128 channels/core = full SBUF partitions
SCALE = (1.0 / NDIM) ** 0.5
DEC = 4                   # decimation factor
J = SEQ_LEN // DEC        # decimated length 1024
CH = 512                  # matmul chunk (one fp32 PSUM bank)
NG = J // CH              # j-groups per slab (2)
F32 = mybir.dt.float32
BF16 = mybir.dt.bfloat16
AF = mybir.ActivationFunctionType
ALU = mybir.AluOpType
NBLK = 4                  # x phase blocks r=0..3 -> x[4j+r]
NWARM = 6                 # HAM warm-up matmuls

# coef columns (host precomputed, fp32), see make_in_maps
NCOEF = 20
CST_W = 2 * NCOEF + D_PER  # merged const tensor width in bf16 units


def build_bass():
    nc = bacc.Bacc(name="multihead_ema_v3")
    x = nc.dram_tensor("x", [D_PER, BSZ, NBLK, J], BF16, kind="ExternalInput")
    cst = nc.dram_tensor("cst", [D_PER, CST_W], BF16, kind="ExternalInput")
    out = nc.dram_tensor("out", [D_PER, BSZ, DEC, J], BF16, kind="ExternalOutput")

    with tile.TileContext(nc) as tc:
        with (
            tc.tile_pool(name="const", bufs=1) as const,
            tc.tile_pool(name="xup", bufs=4) as xup,
            tc.tile_pool(name="yp", bufs=2) as yp,
            tc.tile_pool(name="r0p", bufs=2) as r0p,
            tc.tile_pool(name="op", bufs=2) as op,
            tc.tile_pool(name="psu", bufs=4, space="PSUM") as psu,
            tc.tile_pool(name="ps12", bufs=1, space="PSUM") as ps12,
            tc.tile_pool(name="ps3", bufs=1, space="PSUM") as ps3,
        ):
            # --- warm-up operands (memset on gpsimd; no DMA dependency)
            wz = const.tile([D_PER, D_PER], BF16, tag="wz")
            nc.gpsimd.memset(wz[:, :], 0)
            mz = const.tile([D_PER, CH], BF16, tag="mz")
            nc.gpsimd.memset(mz[:, :], 0)

            # --- input DMAs: const on sync; first x half on scalar queue
            cstsb = const.tile([D_PER, CST_W], BF16)
            nc.sync.dma_start(out=cstsb[:, :], in_=cst[:, :])
            csb = cstsb[:, 0 : 2 * NCOEF].bitcast(F32)   # [128, NCOEF] fp32
            eyesb = cstsb[:, 2 * NCOEF : CST_W]          # [128, 128] bf16
            xus = []
            for b in range(BSZ):
                xu = xup.tile([D_PER, NBLK, J], BF16, tag="xu")
                if b == 0:
                    nc.scalar.dma_start(out=xu[:, :, 0:CH], in_=x[:, b, :, 0:CH])
                    nc.sync.dma_start(out=xu[:, :, CH:J], in_=x[:, b, :, CH:J])
                else:
                    nc.sync.dma_start(out=xu[:, :, :], in_=x[:, b, :, :])
                xus.append(xu)

            # --- HAM warm-up: one accumulation group of dummy matmuls
            wps = psu.tile([D_PER, CH], F32, tag="u")
            for i in range(NWARM):
                nc.tensor.matmul(wps[:, :], wz[:, :], mz[:, :],
                                 start=(i == 0), stop=(i == NWARM - 1))

            # --- diag weight matrices: u-diags on DVE, out-diags on ACT
            def diag_dve(col, tg):
                t = const.tile([D_PER, D_PER], BF16, tag=tg)
                nc.vector.tensor_scalar_mul(out=t[:, :], in0=eyesb[:, :],
                                            scalar1=csb[:, col : col + 1])
                return t

            def diag_act(col, tg):
                t = const.tile([D_PER, D_PER], BF16, tag=tg)
                nc.scalar.activation(out=t[:, :], in_=eyesb[:, :], func=AF.Copy,
                                     scale=csb[:, col : col + 1])
                return t

            # u-synthesis weights, ordered by first use (n=0 taps, n=1 taps)
            w_u = [[None] * 4, [None] * 4]
            for n in range(NDIM):
                for k in range(4):
                    w_u[n][k] = diag_dve(2 + 2 * k + n, f"wu{n}{k}")
            # out-stage weights, ordered by first use in outg
            w_q = [[None] * NDIM for _ in range(3)]  # [r-1][n] : diag(q_n^r)
            w_q[0][0] = diag_act(10, "wq10")
            w_q[0][1] = diag_act(11, "wq11")
            w_cw = diag_act(17, "wcw")    # csum
            w_q[1][0] = diag_act(12, "wq20")
            w_q[1][1] = diag_act(13, "wq21")
            w_cqs = diag_act(18, "wcqs")
            w_q[2][0] = diag_act(14, "wq30")
            w_q[2][1] = diag_act(15, "wq31")
            w_cq2s = diag_act(19, "wcq2s")

            q4b = [csb[:, n : n + 1].to_broadcast([D_PER, CH]) for n in range(NDIM)]

            pus = {}   # (b, n, g) -> psum half tile
            Ys = {}    # (b, n) -> sbuf bf16 tile
            obs = {}   # b -> output tile

            def issue_u(b, n):
                """8 diagonal matmuls accumulating u_n for batch b into two
                single-bank PSUM tiles."""
                xu = xus[b]
                for g in range(NG):
                    pu = psu.tile([D_PER, CH], F32, tag="u")
                    pus[(b, n, g)] = pu
                    s = bass.ts(g, CH)
                    nc.tensor.matmul(pu[:, :], w_u[n][0][:, :], xu[:, 0, s],
                                     start=True, stop=False)
                    for k in range(1, 4):  # + c_n q^k * x[4j-k]
                        if g == 0:
                            nc.tensor.matmul(
                                pu[:, 1:CH], w_u[n][k][:, :],
                                xu[:, 4 - k, 0 : CH - 1],
                                start=False, stop=(k == 3))
                        else:
                            nc.tensor.matmul(
                                pu[:, :], w_u[n][k][:, :],
                                xu[:, 4 - k, g * CH - 1 : (g + 1) * CH - 1],
                                start=False, stop=(k == 3))

            def issue_scan(b, n):
                """Chained half scans: Y'_n available per 512-col half."""
                yn = yp.tile([D_PER, J], BF16, tag=f"y{n}")
                Ys[(b, n)] = yn
                nc.vector.tensor_tensor_scan(
                    out=yn[:, 0:CH], data0=q4b[n],
                    data1=pus.pop((b, n, 0))[:, :],
                    initial=0.0, op0=ALU.mult, op1=ALU.add)
                nc.vector.tensor_tensor_scan(
                    out=yn[:, CH:J], data0=q4b[n],
                    data1=pus.pop((b, n, 1))[:, :],
                    initial=yn[:, CH - 1 : CH], op0=ALU.mult, op1=ALU.add)

            def issue_r0(b):
                """Phase 0 fully off the tensor engine:
                out_0 = silu(Y'0 + Y'1 + w*x0), assembled in bf16 on DVE."""
                ob = op.tile([D_PER, DEC, J], BF16)
                obs[b] = ob
                ysum = r0p.tile([D_PER, J], BF16, tag="ys")
                nc.vector.tensor_add(out=ysum[:, :], in0=Ys[(b, 0)][:, :],
                                     in1=Ys[(b, 1)][:, :])
                xw = r0p.tile([D_PER, J], BF16, tag="xw")
                nc.vector.tensor_scalar_mul(out=xw[:, :], in0=xus[b][:, 0, :],
                                            scalar1=csb[:, 16:17])
                pre0 = r0p.tile([D_PER, J], BF16, tag="p0")
                nc.vector.tensor_add(out=pre0[:, :], in0=ysum[:, :], in1=xw[:, :])
                nc.scalar.activation(out=ob[:, 0, :], in_=pre0[:, :], func=AF.Silu)
                nc.sync.dma_start(out=out[:, b, 0, :], in_=ob[:, 0, :])

            def issue_outg(b, g, split_dma):
                """Phases 1..3 for j-group g: 12 diag matmuls into per-bank
                PSUM tiles, silu evacuation, output DMA."""
                xu = xus[b]
                s = bass.ts(g, CH)
                pt12 = ps12.tile([D_PER, 2, CH], F32, tag="c12")
                pt3 = ps3.tile([D_PER, CH], F32, tag="c3")
                for r in (1, 2, 3):
                    tgt = pt3[:, :] if r == 3 else pt12[:, r - 1, :]
                    nc.tensor.matmul(tgt, w_q[r - 1][0][:, :], Ys[(b, 0)][:, s],
                                     start=True, stop=False)
                    nc.tensor.matmul(tgt, w_q[r - 1][1][:, :], Ys[(b, 1)][:, s],
                                     start=False, stop=False)
                    xw = [(w_cw, r)]
                    if r == 2:
                        xw.append((w_cqs, 1))
                    elif r == 3:
                        xw.append((w_cqs, 2))
                        xw.append((w_cq2s, 1))
                    for i, (wt, rr) in enumerate(xw):
                        nc.tensor.matmul(tgt, wt[:, :], xu[:, rr, s],
                                         start=False, stop=(i == len(xw) - 1))
                nc.scalar.activation(out=obs[b][:, 1:3, s], in_=pt12[:, :, :],
                                     func=AF.Silu)
                nc.scalar.activation(out=obs[b][:, 3, s], in_=pt3[:, :],
                                     func=AF.Silu)
                if split_dma:
                    nc.sync.dma_start(out=out[:, b, 1:3, s],
                                      in_=obs[b][:, 1:3, s])
                    nc.sync.dma_start(out=out[:, b, 3, s], in_=obs[b][:, 3, s])
                else:
                    nc.sync.dma_start(out=out[:, b, 1:4, s],
                                      in_=obs[b][:, 1:4, s])

            # --- software-pipelined main loop
            issue_u(0, 0)
            issue_scan(0, 0)
            issue_u(0, 1)
            issue_scan(0, 1)
            issue_r0(0)
            for b in range(BSZ):
                last = b == BSZ - 1
                if not last:
                    issue_u(b + 1, 0)
                    issue_scan(b + 1, 0)
                issue_outg(b, 0, split_dma=last)
                if not last:
                    issue_u(b + 1, 1)
                    issue_scan(b + 1, 1)
                issue_outg(b, 1, split_dma=last)
                if not last:
                    issue_r0(b + 1)

    nc.compile()
    return nc


_CACHE: dict = {}


def _get_nc():
    if "nc" not in _CACHE:
        _CACHE["nc"] = build_bass()
    return _CACHE["nc"]


def _sigmoid64(a):
    return 1.0 / (1.0 + np.exp(-a.astype(np.float64)))


def make_in_maps(inputs):
    x = np.asarray(inputs["x"], np.float32)
    delta = np.asarray(inputs["delta"], np.float32).reshape(EMBED_DIM, NDIM)
    alpha = np.asarray(inputs["alpha"], np.float32).reshape(EMBED_DIM, NDIM)
    beta = np.asarray(inputs["beta"], np.float32).reshape(EMBED_DIM, NDIM)
    gamma = np.asarray(inputs["gamma"], np.float32).reshape(EMBED_DIM, NDIM)
    omega = np.asarray(inputs["omega"], np.float32).reshape(EMBED_DIM, 1)

    # per-channel coefficient columns (host precompute = weight repacking):
    #  0:2 q^4 | 2:4 c | 4:6 cq | 6:8 cq^2 | 8:10 cq^3 | 10:12 q | 12:14 q^2
    #  14:16 q^3 | 16 w | 17 csum | 18 cqs | 19 cq2s
    p = _sigmoid64(delta)
    q = 1.0 - p * _sigmoid64(alpha)                      # [D, N] f64
    cc = p * beta.astype(np.float64) * gamma.astype(np.float64) * SCALE
    cols = np.empty((EMBED_DIM, NCOEF), np.float64)
    cols[:, 0:2] = q ** 4
    cols[:, 2:4] = cc
    cols[:, 4:6] = cc * q
    cols[:, 6:8] = cc * q ** 2
    cols[:, 8:10] = cc * q ** 3
    cols[:, 10:12] = q
    cols[:, 12:14] = q ** 2
    cols[:, 14:16] = q ** 3
    cols[:, 16:17] = omega
    cols[:, 17:18] = cc.sum(1, keepdims=True) + omega
    cols[:, 18:19] = (cc * q).sum(1, keepdims=True)
    cols[:, 19:20] = (cc * q ** 2).sum(1, keepdims=True)
    coef_full = np.ascontiguousarray(cols.astype(np.float32))

    eye = np.eye(D_PER, dtype=ml_dtypes.bfloat16)
    in_maps = []
    for c in range(N_CORES):
        sl = slice(c * D_PER, (c + 1) * D_PER)
        xc = x[:, :, sl].transpose(2, 1, 0).astype(ml_dtypes.bfloat16)  # [128,B,L]
        ph = xc.reshape(D_PER, BSZ, J, DEC).transpose(0, 1, 3, 2)  # [128,B,4,J]
        cst = np.empty((D_PER, CST_W), dtype=ml_dtypes.bfloat16)
        cst[:, 0 : 2 * NCOEF] = coef_full[sl].view(ml_dtypes.bfloat16)
        cst[:, 2 * NCOEF :] = eye
        in_maps.append({"x": np.ascontiguousarray(ph), "cst": cst})
    return in_maps


def gather_out(results):
    out = np.empty((SEQ_LEN, BSZ, EMBED_DIM), np.float32)
    for c in range(N_CORES):
        # [128, B, 4, J] phase-major -> [l = 4j+r, b, d]
        arr = results[c]["out"].astype(np.float32)
        out[:, :, c * D_PER : (c + 1) * D_PER] = arr.transpose(3, 2, 1, 0).reshape(
            SEQ_LEN, BSZ, D_PER
        )
    return out


def _run(inputs, **kwargs):
    nc = _get_nc()
    in_maps = make_in_maps(inputs)
    res = run_bass_kernel_spmd(nc, in_maps, core_ids=list(range(N_CORES)), **kwargs)
    return gather_out(res.results), res


def kernel(**inputs) -> np.ndarray:
    out, _ = _run(inputs)
    return out
